# revision 1
# baseline (speedup 1.0000x reference)
"""Trainium2 Bass kernel for nn_ClustGeoNodeEncoder (segment_reduce).

Strategy (data-parallel over the cluster axis, per the sharding hint):
  - Host packs the voxel features as 8-f32 rows: x, y, z, value,
    onehot(sem==1..4); count of sem==0 is recovered as n - sum(oh1..4).
  - Clusters are sorted by length and dealt round-robin to the 8 cores so
    every core compiles the same program (SPMD): 32 tiles x 128 clusters
    per core, tile t padded to Lb[t] = max length in its global rank range.
  - The host materializes the per-core dense element stream
    gst[P, S*8] (cluster p's elements for tile t at columns
    off_t*8:(off_t+lb)*8, padded slots all-zero).  The device streams it
    with plain contiguous DMAs -- no per-row indirect gathers.  (SWDGE
    indirect DMA tops out at 128 descriptors / ~1us per instruction,
    which lower-bounds any on-device row gather at ~4.3 ms/core here.)
  - Elementwise work is split between the DVE (vector) engine and the
    Scalar (ACT) engine: ACT does all squared terms (Act.Square with
    accum_out for the diagonal moments), the centering (Identity with a
    per-partition bias of -center), and the square roots.
  - Pass A (per tile): raw sums / second moments / value stats / semantic
    counts via strided tensor_reduce and fused multiply-accumulate;
    centered coordinates are retained in SBUF.
  - Batched per-cluster math on [128, NT] tiles: closed-form symmetric 3x3
    eigenvalues (trig method via Arctan/Sin on the scalar engine),
    principal eigenvector via the spectral projector (A - w0)(A - w1),
    B = A / w2, dirwt = 1 - w1/w2, mode via argmax scan.
  - Pass B (per tile): orientation statistic sc = sum(t * |xc_perp|) from
    the retained centered coords; padded slots contribute a closed-form
    correction term.  Sign-flip + dirwt scaling, then 19 output planes are
    DMA'd out and decoded on the host.
"""

import sys

for _p in ("/opt/trn_rl_repo",):
    if _p not in sys.path:
        sys.path.insert(0, _p)

import numpy as np

N = 2_000_000
C = 32768
L = 256
N_CORES = 8
P = 128
NT = C // (P * N_CORES)  # 32 tiles per core
f32 = np.float32

_PI = float(np.pi)


def _host_prep(data, clust_idx, clust_len):
    data = np.asarray(data, dtype=f32)
    clust_idx = np.asarray(clust_idx).astype(np.int64)
    lens = np.asarray(clust_len).astype(np.int64)

    table = np.zeros((N + 1, 8), dtype=f32)
    table[:N, 0:3] = data[:, 0:3]
    table[:N, 3] = data[:, 4]
    sem = data[:, 5].astype(np.int32)
    for k in range(1, 5):
        table[:N, 3 + k] = (sem == k)

    order = np.argsort(lens, kind="stable")  # ascending length
    # global rank r: tile t = r // (P * N_CORES); slot s = r % (P * N_CORES)
    # core = s % N_CORES ; partition = s // N_CORES
    Lb = np.zeros(NT, dtype=np.int64)
    for t in range(NT):
        Lb[t] = lens[order[t * P * N_CORES:(t + 1) * P * N_CORES]].max()
    S = int(Lb.sum())

    # padded index matrix [C, L] with invalid slots -> N (zero row)
    ar = np.arange(L)[None, :]
    idx_pad = np.where(ar < lens[:, None], clust_idx, N)

    gst = np.zeros((N_CORES, P, S * 8), dtype=f32)
    nvecs = np.zeros((N_CORES, P, NT), dtype=f32)
    ids = np.zeros((N_CORES, NT, P), dtype=np.int64)
    off = 0
    for t in range(NT):
        base = t * P * N_CORES
        lb = int(Lb[t])
        for core in range(N_CORES):
            sel = order[base + core + N_CORES * np.arange(P)]
            ids[core, t] = sel
            nvecs[core, :, t] = lens[sel]
            gst[core, :, off * 8:(off + lb) * 8] = (
                table[idx_pad[sel, :lb]].reshape(P, lb * 8))
        off += lb
    return gst, nvecs, Lb, S, ids


def _build_program(Lb, S):
    import concourse.bass as bass
    import concourse.bacc as bacc
    import concourse.mybir as mybir
    from concourse.tile import TileContext

    dt = mybir.dt
    Alu = mybir.AluOpType
    Act = mybir.ActivationFunctionType

    nc = bacc.Bacc("TRN2", target_bir_lowering=False, debug=False,
                   enable_asserts=False)
    gst = nc.dram_tensor("gst", [P, S * 8], dt.float32, kind="ExternalInput")
    nvec_d = nc.dram_tensor("nvec", [P, NT], dt.float32, kind="ExternalInput")
    res = nc.dram_tensor("res", [P, 19 * NT], dt.float32, kind="ExternalOutput")

    TINY = 1e-30

    with TileContext(nc) as tc:
        with tc.tile_pool(name="ret", bufs=1) as ret, \
             tc.tile_pool(name="gp", bufs=4) as gp, \
             tc.tile_pool(name="sp", bufs=2) as sp, \
             tc.tile_pool(name="sq", bufs=2) as sq:

            def full_tile(tag, k=1):
                return ret.tile([P, k * NT], dt.float32, tag=tag, name=tag)

            NV = full_tile("NV")
            RN = full_tile("RN")
            SUMS = full_tile("SUMS", 4)
            OH = full_tile("OH", 4)
            PROD = full_tile("PROD", 7)
            CEN = full_tile("CEN", 3)
            NCEN = full_tile("NCEN", 3)
            SCRAW = full_tile("SCRAW")
            MEANV = full_tile("MEANV"); STDV = full_tile("STDV")
            MODE = full_tile("MODE")
            B6 = full_tile("B6", 6)
            V3 = full_tile("V3", 3)

            nc.sync.dma_start(out=NV[:], in_=nvec_d[:, :])
            nc.vector.reciprocal(RN[:], NV[:])

            def tt(op, out, a, b):
                nc.vector.tensor_tensor(out=out, in0=a, in1=b, op=op)

            def ts(out, in0, s, op):
                nc.vector.tensor_scalar(out=out, in0=in0, scalar1=s,
                                        scalar2=None, op0=op)

            def stt(out, in0, s, op0, op1, in1, accum=None):
                nc.vector.scalar_tensor_tensor(out=out, in0=in0, scalar=s,
                                               in1=in1, op0=op0, op1=op1,
                                               accum_out=accum)

            def act(out, in_, func, bias=0.0, scale=1.0, accum=None):
                nc.scalar.activation(out, in_, func, bias=bias, scale=scale,
                                     accum_out=accum)

            xcs = []
            offs = []
            off = 0
            for t in range(NT):
                offs.append(off)
                off += int(Lb[t])

            def load_and_pass_a(t):
                lb = int(Lb[t])
                G = gp.tile([P, lb * 8], dt.float32, tag="G", name=f"G{t}")
                nc.sync.dma_start(
                    out=G[:], in_=gst[:, offs[t] * 8:(offs[t] + lb) * 8])
                Gf = G[:].rearrange("p (l f) -> p f l", f=8)
                nc.vector.tensor_reduce(
                    out=SUMS[:].rearrange("p (f t) -> p f t", t=NT)[:, :, t],
                    in_=Gf[:, 0:4, :], axis=mybir.AxisListType.X, op=Alu.add)
                nc.vector.tensor_reduce(
                    out=OH[:].rearrange("p (f t) -> p f t", t=NT)[:, :, t],
                    in_=Gf[:, 4:8, :], axis=mybir.AxisListType.X, op=Alu.add)
                # diagonal moments xx, yy, zz, vv on the scalar engine
                sqs = sq.tile([P, 4 * lb], dt.float32, tag="sqs", name=f"sqs{t}")
                for q, i in ((0, 0), (3, 1), (5, 2), (6, 3)):
                    act(sqs[:, i * lb:(i + 1) * lb], Gf[:, i, :], Act.Square,
                        accum=PROD[:, q * NT + t:q * NT + t + 1])
                # cross moments xy, xz, yz on DVE
                scratch = sp.tile([P, lb], dt.float32, tag="scr", name=f"scr{t}")
                for q, (i, j) in ((1, (0, 1)), (2, (0, 2)), (4, (1, 2))):
                    nc.vector.scalar_tensor_tensor(
                        out=scratch[:],
                        in0=Gf[:, i, :], scalar=1.0, in1=Gf[:, j, :],
                        op0=Alu.mult, op1=Alu.mult,
                        accum_out=PROD[:, q * NT + t:q * NT + t + 1])
                # center and its negation (bias for the ACT centering)
                nc.vector.tensor_scalar(
                    out=NCEN[:].rearrange("p (f t) -> p f t", t=NT)[:, :, t],
                    in0=SUMS[:].rearrange("p (f t) -> p f t", t=NT)[:, 0:3, t],
                    scalar1=RN[:, t:t + 1], scalar2=-1.0,
                    op0=Alu.mult, op1=Alu.mult)
                ts(CEN[:].rearrange("p (f t) -> p f t", t=NT)[:, :, t],
                   NCEN[:].rearrange("p (f t) -> p f t", t=NT)[:, :, t],
                   -1.0, Alu.mult)
                # centered coords on the scalar engine: xc = x + (-c)
                xc = ret.tile([P, 3 * lb], dt.float32, tag=f"xc{t}", name=f"xc{t}")
                for i in range(3):
                    act(xc[:, i * lb:(i + 1) * lb], Gf[:, i, :], Act.Identity,
                        bias=NCEN[:, i * NT + t:i * NT + t + 1])
                xcs.append(xc)

            def cluster_math():
                def tmp(tag, k=1):
                    return ret.tile([P, k * NT], dt.float32, tag=tag, name=tag)

                def sl(T, i):
                    return T[:, i * NT:(i + 1) * NT]

                A = tmp("A", 6)
                cmap = [(0, 0, 0), (1, 0, 1), (2, 0, 2), (3, 1, 1), (4, 1, 2),
                        (5, 2, 2)]
                SC1 = tmp("SC1")
                for q, i, j in cmap:
                    tt(Alu.mult, SC1[:], sl(CEN, i), sl(SUMS, j))
                    tt(Alu.subtract, sl(A, q), sl(PROD, q), SC1[:])

                # value stats
                VAR = tmp("VAR"); NM1 = tmp("NM1")
                tt(Alu.mult, MEANV[:], sl(SUMS, 3), RN[:])
                tt(Alu.mult, VAR[:], MEANV[:], sl(SUMS, 3))
                tt(Alu.subtract, VAR[:], sl(PROD, 6), VAR[:])
                ts(NM1[:], NV[:], 1.0, Alu.subtract)
                nc.vector.reciprocal(SC1[:], NM1[:])
                tt(Alu.mult, VAR[:], VAR[:], SC1[:])
                ts(VAR[:], VAR[:], 0.0, Alu.max)
                act(STDV[:], VAR[:], Act.Sqrt)

                BEST = tmp("BEST"); GT = tmp("GT"); KT = tmp("KT")

                # eigenvalues: trig closed form
                Q = tmp("Q"); P1 = tmp("P1"); P2 = tmp("P2"); PP = tmp("PP")
                RP = tmp("RP"); DET = tmp("DET"); RR = tmp("RR"); SS = tmp("SS")
                AT = tmp("AT"); PHI = tmp("PHI")
                W0 = tmp("W0"); W1 = tmp("W1"); W2 = tmp("W2"); RW2 = tmp("RW2")
                DIRWT = tmp("DIRWT")
                NB = tmp("NB", 6)

                tt(Alu.add, Q[:], sl(A, 0), sl(A, 3))
                tt(Alu.add, Q[:], Q[:], sl(A, 5))
                ts(Q[:], Q[:], 1.0 / 3.0, Alu.mult)

                tt(Alu.mult, P1[:], sl(A, 1), sl(A, 1))
                tt(Alu.mult, SC1[:], sl(A, 2), sl(A, 2))
                tt(Alu.add, P1[:], P1[:], SC1[:])
                tt(Alu.mult, SC1[:], sl(A, 4), sl(A, 4))
                tt(Alu.add, P1[:], P1[:], SC1[:])

                BD = tmp("BD", 3)
                tt(Alu.subtract, sl(BD, 0), sl(A, 0), Q[:])
                tt(Alu.subtract, sl(BD, 1), sl(A, 3), Q[:])
                tt(Alu.subtract, sl(BD, 2), sl(A, 5), Q[:])
                tt(Alu.mult, P2[:], sl(BD, 0), sl(BD, 0))
                tt(Alu.mult, SC1[:], sl(BD, 1), sl(BD, 1))
                tt(Alu.add, P2[:], P2[:], SC1[:])
                tt(Alu.mult, SC1[:], sl(BD, 2), sl(BD, 2))
                tt(Alu.add, P2[:], P2[:], SC1[:])
                stt(P2[:], P1[:], 2.0, Alu.mult, Alu.add, P2[:])
                ts(PP[:], P2[:], 1.0 / 6.0, Alu.mult)
                act(PP[:], PP[:], Act.Sqrt)
                ts(SC1[:], PP[:], TINY, Alu.max)
                nc.vector.reciprocal(RP[:], SC1[:])

                tt(Alu.mult, sl(NB, 0), sl(BD, 0), RP[:])
                tt(Alu.mult, sl(NB, 1), sl(A, 1), RP[:])
                tt(Alu.mult, sl(NB, 2), sl(A, 2), RP[:])
                tt(Alu.mult, sl(NB, 3), sl(BD, 1), RP[:])
                tt(Alu.mult, sl(NB, 4), sl(A, 4), RP[:])
                tt(Alu.mult, sl(NB, 5), sl(BD, 2), RP[:])

                SC2 = tmp("SC2"); SC3 = tmp("SC3")
                tt(Alu.mult, SC1[:], sl(NB, 3), sl(NB, 5))
                tt(Alu.mult, SC2[:], sl(NB, 4), sl(NB, 4))
                tt(Alu.subtract, SC1[:], SC1[:], SC2[:])
                tt(Alu.mult, DET[:], sl(NB, 0), SC1[:])
                tt(Alu.mult, SC1[:], sl(NB, 1), sl(NB, 5))
                tt(Alu.mult, SC2[:], sl(NB, 4), sl(NB, 2))
                tt(Alu.subtract, SC1[:], SC1[:], SC2[:])
                tt(Alu.mult, SC1[:], sl(NB, 1), SC1[:])
                tt(Alu.subtract, DET[:], DET[:], SC1[:])
                tt(Alu.mult, SC1[:], sl(NB, 1), sl(NB, 4))
                tt(Alu.mult, SC2[:], sl(NB, 3), sl(NB, 2))
                tt(Alu.subtract, SC1[:], SC1[:], SC2[:])
                tt(Alu.mult, SC1[:], sl(NB, 2), SC1[:])
                tt(Alu.add, DET[:], DET[:], SC1[:])

                ts(RR[:], DET[:], 0.5, Alu.mult)
                ts(RR[:], RR[:], -1.0, Alu.max)
                ts(RR[:], RR[:], 1.0, Alu.min)
                tt(Alu.mult, SS[:], RR[:], RR[:])
                nc.vector.tensor_scalar(out=SS[:], in0=SS[:], scalar1=-1.0,
                                        scalar2=1.0, op0=Alu.mult, op1=Alu.add)
                ts(SS[:], SS[:], 0.0, Alu.max)
                act(SS[:], SS[:], Act.Sqrt)
                UA = tmp("UA"); UB = tmp("UB")
                ts(SC1[:], RR[:], -1.0, Alu.mult)
                tt(Alu.max, SC1[:], SC1[:], RR[:])
                ts(SS[:], SS[:], TINY, Alu.max)
                nc.vector.reciprocal(SC2[:], SS[:])
                tt(Alu.mult, UA[:], SC1[:], SC2[:])
                ts(SC1[:], UA[:], TINY, Alu.max)
                nc.vector.reciprocal(UB[:], SC1[:])
                tt(Alu.min, SC2[:], UA[:], UB[:])
                act(SC2[:], SC2[:], Act.Arctan)
                ts(SC1[:], UA[:], 1.0, Alu.is_gt)
                nc.vector.tensor_scalar(out=SC3[:], in0=SC2[:], scalar1=-2.0,
                                        scalar2=_PI / 2.0, op0=Alu.mult,
                                        op1=Alu.add)
                tt(Alu.mult, SC3[:], SC3[:], SC1[:])
                tt(Alu.add, SC2[:], SC2[:], SC3[:])
                ts(SC3[:], RR[:], 0.0, Alu.is_lt)
                nc.vector.tensor_scalar(out=SC3[:], in0=SC3[:], scalar1=-2.0,
                                        scalar2=1.0, op0=Alu.mult, op1=Alu.add)
                tt(Alu.mult, AT[:], SC2[:], SC3[:])
                nc.vector.tensor_scalar(out=PHI[:], in0=AT[:],
                                        scalar1=-1.0 / 3.0,
                                        scalar2=_PI / 6.0 + _PI / 2.0,
                                        op0=Alu.mult, op1=Alu.add)
                act(SC1[:], PHI[:], Act.Sin)
                tt(Alu.mult, SC1[:], SC1[:], PP[:])
                stt(W2[:], SC1[:], 2.0, Alu.mult, Alu.add, Q[:])
                nc.vector.tensor_scalar(out=PHI[:], in0=AT[:],
                                        scalar1=-1.0 / 3.0,
                                        scalar2=_PI / 6.0 + _PI / 6.0,
                                        op0=Alu.mult, op1=Alu.add)
                act(SC1[:], PHI[:], Act.Sin)
                tt(Alu.mult, SC1[:], SC1[:], PP[:])
                stt(W0[:], SC1[:], -2.0, Alu.mult, Alu.add, Q[:])
                ts(SC1[:], Q[:], 3.0, Alu.mult)
                tt(Alu.subtract, W1[:], SC1[:], W0[:])
                tt(Alu.subtract, W1[:], W1[:], W2[:])

                ts(SC1[:], W2[:], TINY, Alu.max)
                nc.vector.reciprocal(RW2[:], SC1[:])
                tt(Alu.mult, DIRWT[:], W1[:], RW2[:])
                nc.vector.tensor_scalar(out=DIRWT[:], in0=DIRWT[:],
                                        scalar1=-1.0, scalar2=1.0,
                                        op0=Alu.mult, op1=Alu.add)
                for q in range(6):
                    tt(Alu.mult, sl(B6, q), sl(A, q), RW2[:])

                CD = tmp("CD", 3)
                DD = tmp("DD", 3)
                for qi, ai in enumerate((0, 3, 5)):
                    tt(Alu.subtract, sl(CD, qi), sl(A, ai), W0[:])
                    tt(Alu.subtract, sl(DD, qi), sl(A, ai), W1[:])
                M9 = tmp("M9", 9)

                def mcol(colq, dv):
                    crow = [(sl(CD, 0), sl(A, 1), sl(A, 2)),
                            (sl(A, 1), sl(CD, 1), sl(A, 4)),
                            (sl(A, 2), sl(A, 4), sl(CD, 2))]
                    for r in range(3):
                        a0, a1, a2 = crow[r]
                        tt(Alu.mult, SC1[:], a0, dv[0])
                        tt(Alu.mult, SC2[:], a1, dv[1])
                        tt(Alu.add, SC1[:], SC1[:], SC2[:])
                        tt(Alu.mult, SC2[:], a2, dv[2])
                        tt(Alu.add, sl(M9, colq * 3 + r), SC1[:], SC2[:])

                mcol(0, (sl(DD, 0), sl(A, 1), sl(A, 2)))
                mcol(1, (sl(A, 1), sl(DD, 1), sl(A, 4)))
                mcol(2, (sl(A, 2), sl(A, 4), sl(DD, 2)))

                CN = tmp("CN", 3)
                for j in range(3):
                    tt(Alu.mult, sl(CN, j), sl(M9, j * 3), sl(M9, j * 3))
                    tt(Alu.mult, SC1[:], sl(M9, j * 3 + 1), sl(M9, j * 3 + 1))
                    tt(Alu.add, sl(CN, j), sl(CN, j), SC1[:])
                    tt(Alu.mult, SC1[:], sl(M9, j * 3 + 2), sl(M9, j * 3 + 2))
                    tt(Alu.add, sl(CN, j), sl(CN, j), SC1[:])
                NBEST = tmp("NBEST")
                for i in range(3):
                    nc.vector.tensor_copy(out=sl(V3, i), in_=sl(M9, i))
                nc.vector.tensor_copy(out=NBEST[:], in_=sl(CN, 0))
                for j in (1, 2):
                    tt(Alu.is_gt, GT[:], sl(CN, j), NBEST[:])
                    for i in range(3):
                        tt(Alu.subtract, SC1[:], sl(M9, j * 3 + i), sl(V3, i))
                        tt(Alu.mult, SC1[:], SC1[:], GT[:])
                        tt(Alu.add, sl(V3, i), sl(V3, i), SC1[:])
                    tt(Alu.max, NBEST[:], NBEST[:], sl(CN, j))
                ts(SC1[:], NBEST[:], 1e-37, Alu.max)
                act(SC2[:], SC1[:], Act.Sqrt)
                nc.vector.reciprocal(SC2[:], SC2[:])
                for i in range(3):
                    tt(Alu.mult, sl(V3, i), sl(V3, i), SC2[:])

                # mode of semantic class (ties -> smallest)
                tt(Alu.subtract, BEST[:], NV[:], sl(OH, 0))
                for k in (1, 2, 3):
                    tt(Alu.subtract, BEST[:], BEST[:], sl(OH, k))
                nc.vector.memset(MODE[:], 0.0)
                for k in range(1, 5):
                    ck = sl(OH, k - 1)
                    tt(Alu.is_gt, GT[:], ck, BEST[:])
                    nc.vector.tensor_scalar(out=KT[:], in0=MODE[:],
                                            scalar1=-1.0, scalar2=float(k),
                                            op0=Alu.mult, op1=Alu.add)
                    tt(Alu.mult, KT[:], KT[:], GT[:])
                    tt(Alu.add, MODE[:], MODE[:], KT[:])
                    tt(Alu.max, BEST[:], BEST[:], ck)
                return DIRWT

            def pass_b(t):
                lb = int(Lb[t])
                xc = xcs[t]
                xcx = xc[:, 0:lb]; xcy = xc[:, lb:2 * lb]
                xcz = xc[:, 2 * lb:3 * lb]
                T = sp.tile([P, lb], dt.float32, tag="T", name=f"T{t}")
                S2 = sp.tile([P, lb], dt.float32, tag="S2", name=f"S2_{t}")
                S2b = sp.tile([P, lb], dt.float32, tag="S2b", name=f"S2b{t}")
                R = sp.tile([P, lb], dt.float32, tag="R", name=f"R{t}")
                sq3 = sq.tile([P, 3 * lb], dt.float32, tag="sq3", name=f"sq3{t}")
                nc.vector.tensor_scalar(out=T[:], in0=xcx,
                                        scalar1=V3[:, 0 * NT + t:0 * NT + t + 1],
                                        scalar2=None, op0=Alu.mult)
                stt(T[:], xcy, V3[:, 1 * NT + t:1 * NT + t + 1],
                    Alu.mult, Alu.add, T[:])
                stt(T[:], xcz, V3[:, 2 * NT + t:2 * NT + t + 1],
                    Alu.mult, Alu.add, T[:])
                # squares of the centered coords on the scalar engine
                for i, src in enumerate((xcx, xcy, xcz)):
                    act(sq3[:, i * lb:(i + 1) * lb], src, Act.Square)
                tt(Alu.add, S2[:], sq3[:, 0:lb], sq3[:, lb:2 * lb])
                tt(Alu.add, S2[:], S2[:], sq3[:, 2 * lb:3 * lb])
                stt(S2b[:], T[:], -1.0, Alu.mult, Alu.mult, T[:])
                tt(Alu.add, S2[:], S2[:], S2b[:])
                ts(S2[:], S2[:], 0.0, Alu.max)
                act(R[:], S2[:], Act.Sqrt)
                stt(S2b[:], T[:], 1.0, Alu.mult, Alu.mult, R[:],
                    accum=SCRAW[:, t:t + 1])

            def sign_phase(DIRWT):
                def tmp(tag, k=1):
                    return ret.tile([P, k * NT], dt.float32, tag=tag, name=tag)

                def sl(T, i):
                    return T[:, i * NT:(i + 1) * NT]

                T0 = tmp("T0"); CC = tmp("CC"); R0 = tmp("R0")
                SCV = tmp("SCV"); FAC = tmp("FAC"); SC9 = tmp("SC9")
                GT9 = tmp("GT9"); NPAD = tmp("NPAD")
                tt(Alu.mult, T0[:], sl(CEN, 0), sl(V3, 0))
                tt(Alu.mult, SC9[:], sl(CEN, 1), sl(V3, 1))
                tt(Alu.add, T0[:], T0[:], SC9[:])
                tt(Alu.mult, SC9[:], sl(CEN, 2), sl(V3, 2))
                tt(Alu.add, T0[:], T0[:], SC9[:])
                ts(T0[:], T0[:], -1.0, Alu.mult)
                tt(Alu.mult, CC[:], sl(CEN, 0), sl(CEN, 0))
                tt(Alu.mult, SC9[:], sl(CEN, 1), sl(CEN, 1))
                tt(Alu.add, CC[:], CC[:], SC9[:])
                tt(Alu.mult, SC9[:], sl(CEN, 2), sl(CEN, 2))
                tt(Alu.add, CC[:], CC[:], SC9[:])
                tt(Alu.mult, SC9[:], T0[:], T0[:])
                tt(Alu.subtract, R0[:], CC[:], SC9[:])
                ts(R0[:], R0[:], 0.0, Alu.max)
                act(R0[:], R0[:], Act.Sqrt)
                for t in range(NT):
                    nc.vector.tensor_scalar(
                        out=NPAD[:, t:t + 1],
                        in0=NV[:, t:t + 1], scalar1=-1.0,
                        scalar2=float(int(Lb[t])), op0=Alu.mult, op1=Alu.add)
                tt(Alu.mult, SC9[:], T0[:], R0[:])
                tt(Alu.mult, SC9[:], SC9[:], NPAD[:])
                tt(Alu.subtract, SCV[:], SCRAW[:], SC9[:])
                ts(GT9[:], SCV[:], 0.0, Alu.is_lt)
                nc.vector.tensor_scalar(out=GT9[:], in0=GT9[:], scalar1=-2.0,
                                        scalar2=1.0, op0=Alu.mult, op1=Alu.add)
                tt(Alu.mult, FAC[:], DIRWT[:], GT9[:])
                for i in range(3):
                    tt(Alu.mult, sl(V3, i), sl(V3, i), FAC[:])
                for j, pl in [(0, sl(CEN, 0)), (1, sl(CEN, 1)), (2, sl(CEN, 2)),
                              (3, sl(B6, 0)), (4, sl(B6, 1)), (5, sl(B6, 2)),
                              (6, sl(B6, 1)), (7, sl(B6, 3)), (8, sl(B6, 4)),
                              (9, sl(B6, 2)), (10, sl(B6, 4)), (11, sl(B6, 5)),
                              (12, sl(V3, 0)), (13, sl(V3, 1)), (14, sl(V3, 2)),
                              (15, NV[:]), (16, MEANV[:]), (17, STDV[:]),
                              (18, MODE[:])]:
                    nc.sync.dma_start(out=res[:, j * NT:(j + 1) * NT], in_=pl)

            for t in range(NT):
                load_and_pass_a(t)
            DIRWT = cluster_math()
            for t in range(NT):
                pass_b(t)
            sign_phase(DIRWT)

    nc.compile()
    return nc


_cache = {}
_last = None


def kernel(data, clust_idx, clust_len):
    global N, C, L, NT
    data = np.asarray(data)
    clust_idx = np.asarray(clust_idx)
    N = int(data.shape[0])
    C, L = int(clust_idx.shape[0]), int(clust_idx.shape[1])
    assert C % (P * N_CORES) == 0, f"cluster count {C} not divisible by {P * N_CORES}"
    NT = C // (P * N_CORES)
    gst, nvecs, Lb, S, ids = _host_prep(data, clust_idx, clust_len)

    key = tuple(int(x) for x in Lb)
    if key not in _cache:
        _cache[key] = _build_program(Lb, S)
    nc = _cache[key]

    from concourse.bass_utils import run_bass_kernel_spmd
    in_maps = [{"gst": gst[c], "nvec": nvecs[c]} for c in range(N_CORES)]
    global _last
    _last = (nc, in_maps)
    res = run_bass_kernel_spmd(nc, in_maps, list(range(N_CORES)))

    out = np.zeros((C, 19), dtype=f32)
    for core in range(N_CORES):
        r = res.results[core]["res"].reshape(P, 19, NT)
        for t in range(NT):
            out[ids[core, t]] = r[:, :, t]
    return out



# revision 5
# speedup vs baseline: 1.3829x; 1.3829x over previous
"""Trainium2 Bass kernel for nn_ClustGeoNodeEncoder (segment_reduce).

v2 architecture (PE-accelerated moments):
  - Host sorts clusters by length, deals them round-robin to 8 cores x 32
    tiles of 128 clusters (one cluster per partition per tile), and stages
    TWO fp16 streams per core:
      * transposed stream: [128 element-slots, ncols] feature planes
        (x/16, y/16, z/16, v, ca, cb) where each column holds up to 128
        elements of one cluster chunk (2 chunks for tiles padded > 128).
        ca = oh1 + 512*oh2, cb = oh3 + 512*oh4 pack the semantic one-hots
        (exact in fp16; sums stay < 2^24 so fp32 PSUM accumulation is
        exact).  Columns are ordered partition-major so partition p's
        clusters occupy a contiguous 13*NT-column window.
      * cluster-major stream: [128 clusters, S] x/16, y/16, z/16 planes,
        feature-major per group of 4 equal-padded tiles (for pass B).
  - Device pass A: ACT squares the coordinate/value planes, DVE forms the
    three cross-product planes (2x fp16 mode), and the TensorEngine
    reduces all 13 moment planes per cluster with ones-column matmuls:
    a staircase window (ones only in absolute column 128 of a [128, 256]
    buffer) places partition p's sums into PSUM row p; 128 accumulating
    matmuls cover all partitions, long-tile second chunks accumulate into
    the same PSUM columns.  One [128, 13*NT] PSUM->SBUF copy evacuates
    every raw moment.
  - Cluster math on [128, NT] fp32 planes: centers, centered scatter
    matrix A (scale-free in /16 units), closed-form trig eigenvalues,
    principal eigenvector via spectral projector, B = A/w2, dirwt, value
    stats, semantic mode via int32-truncation unpack of ca/cb.
  - Pass B (cluster-major): ts-centering (4x fp16), per-tile stt dot with
    v0, ACT group squares + sqrt, stt-accum orientation statistic sc;
    padded-slot closed-form correction, sign flip, output DMA.
"""

import sys

for _p in ("/opt/trn_rl_repo",):
    if _p not in sys.path:
        sys.path.insert(0, _p)

import numpy as np

N = 2_000_000
C = 32768
L = 256
N_CORES = 8
P = 128
NT = C // (P * N_CORES)  # 32 tiles per core
NG = 8                   # pass-B tile groups (4 tiles each, shared pad)
f32 = np.float32
f16 = np.float16

_PI = float(np.pi)
SCL = 16.0               # coordinate pre-scale (powers of 2 are exact)


def _host_prep(data, clust_idx, clust_len):
    data = np.asarray(data, dtype=f32)
    clust_idx = np.asarray(clust_idx).astype(np.int64)
    lens = np.asarray(clust_len).astype(np.int64)

    # feature table: x/16, y/16, z/16, v, ca, cb ; row N = zeros for padding
    table = np.zeros((N + 1, 6), dtype=f32)
    table[:N, 0:3] = data[:, 0:3] / SCL
    table[:N, 3] = data[:, 4]
    sem = data[:N, 5].astype(np.int32)
    ca = (sem == 1).astype(f32) + 512.0 * (sem == 2)
    cb = (sem == 3).astype(f32) + 512.0 * (sem == 4)
    table[:N, 4] = ca
    table[:N, 5] = cb

    order = np.argsort(lens, kind="stable")
    # rank r: tile t = r // (P*N_CORES); slot s = r % (P*N_CORES)
    # core = s % N_CORES ; partition = s // N_CORES
    Lb = np.zeros(NT, dtype=np.int64)
    for t in range(NT):
        Lb[t] = lens[order[t * P * N_CORES:(t + 1) * P * N_CORES]].max()
    # pass-B groups of 4 tiles share a padded length
    Lg = np.zeros(NG, dtype=np.int64)
    for g in range(NG):
        Lg[g] = Lb[4 * g:4 * g + 4].max()
    Sg = int(Lg.sum() * 4)          # cluster-major columns per partition

    chunks = np.maximum(1, (Lb + 127) // 128)     # 1 or 2 per tile
    n2 = int((chunks == 2).sum())                 # trailing tiles (sorted)
    t2_start = NT - n2
    ncol_p = NT + n2                              # columns per partition
    NCOL = P * ncol_p

    ar = np.arange(L)[None, :]
    idx_pad = np.where(ar < lens[:, None], clust_idx, N)

    ids = np.zeros((N_CORES, NT, P), dtype=np.int64)
    nvecs = np.zeros((N_CORES, P, NT), dtype=f32)
    # transposed stream: [core][128 slots, 6 planes * NCOL] plane-major
    tstr = np.zeros((N_CORES, P, 6 * NCOL), dtype=f16)
    # cluster-major stream: [core][128, 3 * Sg] plane-major, group-padded
    cstr = np.zeros((N_CORES, P, 3 * Sg), dtype=f16)

    goff = np.zeros(NG, dtype=np.int64)
    off = 0
    for g in range(NG):
        goff[g] = off
        off += 4 * int(Lg[g])

    tv = tstr.reshape(N_CORES, P, 6, P, ncol_p)
    cv = cstr.reshape(N_CORES, P, 3, Sg)
    for t in range(NT):
        base = t * P * N_CORES
        g, tg = t // 4, t % 4
        lg = int(Lg[g])
        lb = int(Lb[t])
        c1 = min(lb, 128)
        for core in range(N_CORES):
            sel = order[base + core + N_CORES * np.arange(P)]
            ids[core, t] = sel
            nvecs[core, :, t] = lens[sel]
            feats = table[idx_pad[sel, :lb]]          # [P, lb, 6]
            # transposed: chunk 0 -> col t, chunk 1 -> col NT + (t - t2_start)
            blk = np.zeros((P, 128, 6), dtype=f32)
            blk[:, :c1] = feats[:, :c1]
            tv[core, :, :, :, t] = blk.transpose(1, 2, 0).astype(f16)
            if lb > 128:
                blk2 = np.zeros((P, 128, 6), dtype=f32)
                blk2[:, :lb - 128] = feats[:, 128:lb]
                tv[core, :, :, :, NT + (t - t2_start)] = (
                    blk2.transpose(1, 2, 0).astype(f16))
            # cluster-major x/16,y/16,z/16 planes, group layout
            s0 = int(goff[g]) + tg * lg
            cv[core, :, :, s0:s0 + lb] = (
                feats[:, :, 0:3].transpose(0, 2, 1).astype(f16))
    return dict(tstr=tstr, cstr=cstr, nvecs=nvecs, ids=ids, Lb=Lb, Lg=Lg,
                Sg=Sg, ncol_p=ncol_p, NCOL=NCOL, n2=n2, t2_start=t2_start,
                goff=goff)


def _build_program(meta):
    import concourse.bass as bass
    import concourse.bacc as bacc
    import concourse.mybir as mybir
    from concourse.tile import TileContext

    dt = mybir.dt
    Alu = mybir.AluOpType
    Act = mybir.ActivationFunctionType

    Lb = meta["Lb"]; Lg = meta["Lg"]; Sg = meta["Sg"]
    ncol_p = meta["ncol_p"]; NCOL = meta["NCOL"]
    n2 = meta["n2"]; t2s = meta["t2_start"]; goff = meta["goff"]
    NM = 13                       # moment planes

    nc = bacc.Bacc("TRN2", target_bir_lowering=False, debug=False,
                   enable_asserts=False)
    tstr = nc.dram_tensor("tstr", [P, 6 * NCOL], dt.float16,
                          kind="ExternalInput")
    cstr = nc.dram_tensor("cstr", [P, 3 * Sg], dt.float16,
                          kind="ExternalInput")
    nvec_d = nc.dram_tensor("nvec", [P, NT], dt.float32, kind="ExternalInput")
    res = nc.dram_tensor("res", [P, 19 * NT], dt.float32,
                         kind="ExternalOutput")

    TINY = 1e-30
    HALF = NCOL // 2              # column split for SBUF residency

    with TileContext(nc) as tc:
        with tc.tile_pool(name="ret", bufs=1) as ret, \
             tc.tile_pool(name="ps", bufs=1, space="PSUM") as ps:

            def full_tile(tag, k=1):
                return ret.tile([P, k * NT], dt.float32, tag=tag, name=tag)

            NV = full_tile("NV")
            RN = full_tile("RN")
            RAWM = ret.tile([P, NM * NT], dt.float32, tag="RAWM", name="RAWM")
            SCRAW = full_tile("SCRAW")
            MEANV = full_tile("MEANV"); STDV = full_tile("STDV")
            MODE = full_tile("MODE")
            B6 = full_tile("B6", 6)
            V3 = full_tile("V3", 3)
            CEN = full_tile("CEN", 3)
            NCEN = full_tile("NCEN", 3)
            STAIR = ret.tile([P, 256], dt.float16, tag="STAIR", name="STAIR")

            nc.sync.dma_start(out=NV[:], in_=nvec_d[:, :])
            nc.vector.reciprocal(RN[:], NV[:])
            nc.vector.memset(STAIR[:], 0.0)
            nc.vector.memset(STAIR[:, 128:129], 1.0)

            def tt(op, out, a, b):
                nc.vector.tensor_tensor(out=out, in0=a, in1=b, op=op)

            def ts(out, in0, s, op, s2=None, op1=None):
                kw = {}
                if op1 is not None:
                    kw["op1"] = op1
                nc.vector.tensor_scalar(out=out, in0=in0, scalar1=s,
                                        scalar2=s2, op0=op, **kw)

            def stt(out, in0, s, op0, op1, in1, accum=None):
                nc.vector.scalar_tensor_tensor(out=out, in0=in0, scalar=s,
                                               in1=in1, op0=op0, op1=op1,
                                               accum_out=accum)

            def act(out, in_, func, bias=0.0, scale=1.0, accum=None):
                nc.scalar.activation(out, in_, func, bias=bias, scale=scale,
                                     accum_out=accum)

            # ---------------- Pass A: PE moment sums -----------------
            PS = ps.tile([P, NM * NT], dt.float32, tag="PS", name="PS")
            first = [True]

            def pass_a_half(h, hp, pp):
                c0, c1 = h * HALF, (h + 1) * HALF
                W = c1 - c0
                raw = hp.tile([P, 6 * W], dt.float16, tag="raw",
                              name=f"raw{h}")
                nc.sync.dma_start(
                    out=raw[:].rearrange("k (f c) -> k f c", f=6),
                    in_=tstr[:, :].rearrange("k (f c) -> k f c", f=6)[:, :, c0:c1])
                prod = pp.tile([P, 7 * W], dt.float16, tag="prod",
                               name=f"prod{h}")
                rawv = raw[:].rearrange("k (f c) -> k f c", f=6)
                prodv = prod[:].rearrange("k (f c) -> k f c", f=7)
                # squares of x,y,z,v in one ACT instruction
                act(prod[:, 0:4 * W], raw[:, 0:4 * W], Act.Square)
                # crosses xy, xz, yz on DVE (2x fp16)
                tt(Alu.mult, prodv[:, 4, :], rawv[:, 0, :], rawv[:, 1, :])
                tt(Alu.mult, prodv[:, 5, :], rawv[:, 0, :], rawv[:, 2, :])
                tt(Alu.mult, prodv[:, 6, :], rawv[:, 1, :], rawv[:, 2, :])

                # per-partition matmuls (staircase window -> PSUM row p)
                p0, p1 = c0 // ncol_p, c1 // ncol_p
                psv = PS[:].rearrange("p (m t) -> p m t", m=NM)
                for p in range(p0, p1):
                    rb = p * ncol_p - c0
                    # plane order: x,y,z,v,ca,cb, xx,yy,zz,vv, xy,xz,yz
                    r6 = rawv[:, :, rb:rb + ncol_p]
                    r7 = prodv[:, :, rb:rb + ncol_p]
                    lhs = STAIR[:, 128 - p:256 - p]
                    st = first[0]; first[0] = False
                    nc.tensor.matmul(psv[:, 0:6, 0:NT], lhs, r6[:, :, 0:NT],
                                     start=st, stop=False)
                    nc.tensor.matmul(psv[:, 6:13, 0:NT], lhs, r7[:, :, 0:NT],
                                     start=st, stop=False)
                    last = (p == P - 1)
                    if n2 > 0:
                        nc.tensor.matmul(psv[:, 0:6, t2s:NT], lhs,
                                         r6[:, :, NT:ncol_p],
                                         start=False, stop=False)
                        nc.tensor.matmul(psv[:, 6:13, t2s:NT], lhs,
                                         r7[:, :, NT:ncol_p],
                                         start=False, stop=last)
                    elif last:
                        # close the accumulation group with a zero-contrib
                        # matmul (reads the stair's zero column).
                        nc.tensor.matmul(psv[:, 6:7, NT - 1:NT],
                                         STAIR[:, 0:1].broadcast_to((P, 1)),
                                         STAIR[:, 0:1], start=False, stop=True)

            with tc.tile_pool(name="half", bufs=2) as hp, \
                 tc.tile_pool(name="prod", bufs=2) as pp:
                pass_a_half(0, hp, pp)
                pass_a_half(1, hp, pp)
            nc.vector.tensor_copy(out=RAWM[:], in_=PS[:])

            def msl(m):
                return RAWM[:, m * NT:(m + 1) * NT]

            # ---------------- cluster math ----------------------------
            def cluster_math():
                def tmp(tag, k=1):
                    return ret.tile([P, k * NT], dt.float32, tag=tag, name=tag)

                def sl(T, i):
                    return T[:, i * NT:(i + 1) * NT]

                SC1 = tmp("SC1"); SC2 = tmp("SC2"); SC3 = tmp("SC3")
                # centers (scaled units): c' = sum(x')/n
                for i in range(3):
                    tt(Alu.mult, sl(CEN, i), msl(i), RN[:])
                    ts(sl(NCEN, i), sl(CEN, i), -1.0, Alu.mult)
                # A' = prod - cen*sum  (xx,xy,xz,yy,yz,zz in cmap order)
                A = tmp("A", 6)
                # raw plane order: 6=xx,7=yy,8=zz,9=vv,10=xy,11=xz,12=yz
                pmap = [(0, 6, 0, 0), (1, 10, 0, 1), (2, 11, 0, 2),
                        (3, 7, 1, 1), (4, 12, 1, 2), (5, 8, 2, 2)]
                for q, pm, i, j in pmap:
                    tt(Alu.mult, SC1[:], sl(CEN, i), msl(j))
                    tt(Alu.subtract, sl(A, q), msl(pm), SC1[:])

                # value stats: meanv = sum(v)/n ; var = (sum(v^2)-mean*sum)/ (n-1)
                VAR = tmp("VAR"); NM1 = tmp("NM1")
                tt(Alu.mult, MEANV[:], msl(3), RN[:])
                tt(Alu.mult, VAR[:], MEANV[:], msl(3))
                tt(Alu.subtract, VAR[:], msl(9), VAR[:])
                ts(NM1[:], NV[:], 1.0, Alu.subtract)
                nc.vector.reciprocal(SC1[:], NM1[:])
                tt(Alu.mult, VAR[:], VAR[:], SC1[:])
                ts(VAR[:], VAR[:], 0.0, Alu.max)
                act(STDV[:], VAR[:], Act.Sqrt)

                # unpack semantic counts: ca -> c1 + 512*c2, cb -> c3 + 512*c4
                CNT = tmp("CNT", 4)
                HI_I = ret.tile([P, 2 * NT], dt.int32, tag="HI_I", name="HI_I")
                HIF = tmp("HIF", 2)
                for k, src in ((0, msl(4)), (1, msl(5))):
                    ts(sl(HIF, k), src, 1.0 / 512.0, Alu.mult)
                nc.vector.tensor_copy(out=HI_I[:], in_=HIF[:])
                nc.vector.tensor_copy(out=HIF[:], in_=HI_I[:])
                # c2 = floor(ca/512); c1 = ca - 512*c2
                for k, src in ((0, msl(4)), (1, msl(5))):
                    ts(SC1[:], sl(HIF, k), -512.0, Alu.mult)
                    tt(Alu.add, sl(CNT, 2 * k), src, SC1[:])
                    nc.vector.tensor_copy(out=sl(CNT, 2 * k + 1), in_=sl(HIF, k))

                BEST = tmp("BEST"); GT = tmp("GT"); KT = tmp("KT")
                tt(Alu.subtract, BEST[:], NV[:], sl(CNT, 0))
                for k in (1, 2, 3):
                    tt(Alu.subtract, BEST[:], BEST[:], sl(CNT, k))
                nc.vector.memset(MODE[:], 0.0)
                for k in range(1, 5):
                    ck = sl(CNT, k - 1)
                    tt(Alu.is_gt, GT[:], ck, BEST[:])
                    nc.vector.tensor_scalar(out=KT[:], in0=MODE[:],
                                            scalar1=-1.0, scalar2=float(k),
                                            op0=Alu.mult, op1=Alu.add)
                    tt(Alu.mult, KT[:], KT[:], GT[:])
                    tt(Alu.add, MODE[:], MODE[:], KT[:])
                    tt(Alu.max, BEST[:], BEST[:], ck)

                # eigenvalues: trig closed form on A'
                Q = tmp("Q"); P1 = tmp("P1"); P2 = tmp("P2"); PP = tmp("PP")
                RP = tmp("RP"); DET = tmp("DET"); RR = tmp("RR"); SS = tmp("SS")
                AT = tmp("AT"); PHI = tmp("PHI")
                W0 = tmp("W0"); W1 = tmp("W1"); W2 = tmp("W2"); RW2 = tmp("RW2")
                DIRWT = tmp("DIRWT")
                NB = tmp("NB", 6)

                tt(Alu.add, Q[:], sl(A, 0), sl(A, 3))
                tt(Alu.add, Q[:], Q[:], sl(A, 5))
                ts(Q[:], Q[:], 1.0 / 3.0, Alu.mult)

                tt(Alu.mult, P1[:], sl(A, 1), sl(A, 1))
                tt(Alu.mult, SC1[:], sl(A, 2), sl(A, 2))
                tt(Alu.add, P1[:], P1[:], SC1[:])
                tt(Alu.mult, SC1[:], sl(A, 4), sl(A, 4))
                tt(Alu.add, P1[:], P1[:], SC1[:])

                BD = tmp("BD", 3)
                tt(Alu.subtract, sl(BD, 0), sl(A, 0), Q[:])
                tt(Alu.subtract, sl(BD, 1), sl(A, 3), Q[:])
                tt(Alu.subtract, sl(BD, 2), sl(A, 5), Q[:])
                tt(Alu.mult, P2[:], sl(BD, 0), sl(BD, 0))
                tt(Alu.mult, SC1[:], sl(BD, 1), sl(BD, 1))
                tt(Alu.add, P2[:], P2[:], SC1[:])
                tt(Alu.mult, SC1[:], sl(BD, 2), sl(BD, 2))
                tt(Alu.add, P2[:], P2[:], SC1[:])
                stt(P2[:], P1[:], 2.0, Alu.mult, Alu.add, P2[:])
                ts(PP[:], P2[:], 1.0 / 6.0, Alu.mult)
                act(PP[:], PP[:], Act.Sqrt)
                ts(SC1[:], PP[:], TINY, Alu.max)
                nc.vector.reciprocal(RP[:], SC1[:])

                tt(Alu.mult, sl(NB, 0), sl(BD, 0), RP[:])
                tt(Alu.mult, sl(NB, 1), sl(A, 1), RP[:])
                tt(Alu.mult, sl(NB, 2), sl(A, 2), RP[:])
                tt(Alu.mult, sl(NB, 3), sl(BD, 1), RP[:])
                tt(Alu.mult, sl(NB, 4), sl(A, 4), RP[:])
                tt(Alu.mult, sl(NB, 5), sl(BD, 2), RP[:])

                tt(Alu.mult, SC1[:], sl(NB, 3), sl(NB, 5))
                tt(Alu.mult, SC2[:], sl(NB, 4), sl(NB, 4))
                tt(Alu.subtract, SC1[:], SC1[:], SC2[:])
                tt(Alu.mult, DET[:], sl(NB, 0), SC1[:])
                tt(Alu.mult, SC1[:], sl(NB, 1), sl(NB, 5))
                tt(Alu.mult, SC2[:], sl(NB, 4), sl(NB, 2))
                tt(Alu.subtract, SC1[:], SC1[:], SC2[:])
                tt(Alu.mult, SC1[:], sl(NB, 1), SC1[:])
                tt(Alu.subtract, DET[:], DET[:], SC1[:])
                tt(Alu.mult, SC1[:], sl(NB, 1), sl(NB, 4))
                tt(Alu.mult, SC2[:], sl(NB, 3), sl(NB, 2))
                tt(Alu.subtract, SC1[:], SC1[:], SC2[:])
                tt(Alu.mult, SC1[:], sl(NB, 2), SC1[:])
                tt(Alu.add, DET[:], DET[:], SC1[:])

                ts(RR[:], DET[:], 0.5, Alu.mult)
                ts(RR[:], RR[:], -1.0, Alu.max)
                ts(RR[:], RR[:], 1.0, Alu.min)
                tt(Alu.mult, SS[:], RR[:], RR[:])
                nc.vector.tensor_scalar(out=SS[:], in0=SS[:], scalar1=-1.0,
                                        scalar2=1.0, op0=Alu.mult, op1=Alu.add)
                ts(SS[:], SS[:], 0.0, Alu.max)
                act(SS[:], SS[:], Act.Sqrt)
                UA = tmp("UA"); UB = tmp("UB")
                ts(SC1[:], RR[:], -1.0, Alu.mult)
                tt(Alu.max, SC1[:], SC1[:], RR[:])
                ts(SS[:], SS[:], TINY, Alu.max)
                nc.vector.reciprocal(SC2[:], SS[:])
                tt(Alu.mult, UA[:], SC1[:], SC2[:])
                ts(SC1[:], UA[:], TINY, Alu.max)
                nc.vector.reciprocal(UB[:], SC1[:])
                tt(Alu.min, SC2[:], UA[:], UB[:])
                act(SC2[:], SC2[:], Act.Arctan)
                ts(SC1[:], UA[:], 1.0, Alu.is_gt)
                nc.vector.tensor_scalar(out=SC3[:], in0=SC2[:], scalar1=-2.0,
                                        scalar2=_PI / 2.0, op0=Alu.mult,
                                        op1=Alu.add)
                tt(Alu.mult, SC3[:], SC3[:], SC1[:])
                tt(Alu.add, SC2[:], SC2[:], SC3[:])
                ts(SC3[:], RR[:], 0.0, Alu.is_lt)
                nc.vector.tensor_scalar(out=SC3[:], in0=SC3[:], scalar1=-2.0,
                                        scalar2=1.0, op0=Alu.mult, op1=Alu.add)
                tt(Alu.mult, AT[:], SC2[:], SC3[:])
                nc.vector.tensor_scalar(out=PHI[:], in0=AT[:],
                                        scalar1=-1.0 / 3.0,
                                        scalar2=_PI / 6.0 + _PI / 2.0,
                                        op0=Alu.mult, op1=Alu.add)
                act(SC1[:], PHI[:], Act.Sin)
                tt(Alu.mult, SC1[:], SC1[:], PP[:])
                stt(W2[:], SC1[:], 2.0, Alu.mult, Alu.add, Q[:])
                nc.vector.tensor_scalar(out=PHI[:], in0=AT[:],
                                        scalar1=-1.0 / 3.0,
                                        scalar2=_PI / 6.0 + _PI / 6.0,
                                        op0=Alu.mult, op1=Alu.add)
                act(SC1[:], PHI[:], Act.Sin)
                tt(Alu.mult, SC1[:], SC1[:], PP[:])
                stt(W0[:], SC1[:], -2.0, Alu.mult, Alu.add, Q[:])
                ts(SC1[:], Q[:], 3.0, Alu.mult)
                tt(Alu.subtract, W1[:], SC1[:], W0[:])
                tt(Alu.subtract, W1[:], W1[:], W2[:])

                ts(SC1[:], W2[:], TINY, Alu.max)
                nc.vector.reciprocal(RW2[:], SC1[:])
                tt(Alu.mult, DIRWT[:], W1[:], RW2[:])
                nc.vector.tensor_scalar(out=DIRWT[:], in0=DIRWT[:],
                                        scalar1=-1.0, scalar2=1.0,
                                        op0=Alu.mult, op1=Alu.add)
                for q in range(6):
                    tt(Alu.mult, sl(B6, q), sl(A, q), RW2[:])

                CD = tmp("CD", 3)
                DD = tmp("DD", 3)
                for qi, ai in enumerate((0, 3, 5)):
                    tt(Alu.subtract, sl(CD, qi), sl(A, ai), W0[:])
                    tt(Alu.subtract, sl(DD, qi), sl(A, ai), W1[:])
                M9 = tmp("M9", 9)

                def mcol(colq, dv):
                    crow = [(sl(CD, 0), sl(A, 1), sl(A, 2)),
                            (sl(A, 1), sl(CD, 1), sl(A, 4)),
                            (sl(A, 2), sl(A, 4), sl(CD, 2))]
                    for r in range(3):
                        a0, a1, a2 = crow[r]
                        tt(Alu.mult, SC1[:], a0, dv[0])
                        tt(Alu.mult, SC2[:], a1, dv[1])
                        tt(Alu.add, SC1[:], SC1[:], SC2[:])
                        tt(Alu.mult, SC2[:], a2, dv[2])
                        tt(Alu.add, sl(M9, colq * 3 + r), SC1[:], SC2[:])

                mcol(0, (sl(DD, 0), sl(A, 1), sl(A, 2)))
                mcol(1, (sl(A, 1), sl(DD, 1), sl(A, 4)))
                mcol(2, (sl(A, 2), sl(A, 4), sl(DD, 2)))

                CN = tmp("CN", 3)
                for j in range(3):
                    tt(Alu.mult, sl(CN, j), sl(M9, j * 3), sl(M9, j * 3))
                    tt(Alu.mult, SC1[:], sl(M9, j * 3 + 1), sl(M9, j * 3 + 1))
                    tt(Alu.add, sl(CN, j), sl(CN, j), SC1[:])
                    tt(Alu.mult, SC1[:], sl(M9, j * 3 + 2), sl(M9, j * 3 + 2))
                    tt(Alu.add, sl(CN, j), sl(CN, j), SC1[:])
                NBEST = tmp("NBEST")
                for i in range(3):
                    nc.vector.tensor_copy(out=sl(V3, i), in_=sl(M9, i))
                nc.vector.tensor_copy(out=NBEST[:], in_=sl(CN, 0))
                for j in (1, 2):
                    tt(Alu.is_gt, GT[:], sl(CN, j), NBEST[:])
                    for i in range(3):
                        tt(Alu.subtract, SC1[:], sl(M9, j * 3 + i), sl(V3, i))
                        tt(Alu.mult, SC1[:], SC1[:], GT[:])
                        tt(Alu.add, sl(V3, i), sl(V3, i), SC1[:])
                    tt(Alu.max, NBEST[:], NBEST[:], sl(CN, j))
                ts(SC1[:], NBEST[:], 1e-37, Alu.max)
                act(SC2[:], SC1[:], Act.Sqrt)
                nc.vector.reciprocal(SC2[:], SC2[:])
                for i in range(3):
                    tt(Alu.mult, sl(V3, i), sl(V3, i), SC2[:])
                return DIRWT

            DIRWT = cluster_math()

            # ---------------- pass B (cluster-major, scaled units) ----
            from contextlib import ExitStack
            _pb_stack = ExitStack()
            pbp = _pb_stack.enter_context(tc.tile_pool(name="pbp", bufs=1))
            pb = _pb_stack.enter_context(tc.tile_pool(name="pb", bufs=2))
            CSTR = pbp.tile([P, 3 * Sg], dt.float16, tag="CSTR", name="CSTR")
            nc.sync.dma_start(out=CSTR[:], in_=cstr[:, :])
            XC = pbp.tile([P, 3 * Sg], dt.float16, tag="XC", name="XC")
            TP = pbp.tile([P, Sg], dt.float16, tag="TP", name="TP")

            def cm_plane(i):   # cluster-major input plane i
                return CSTR[:, i * Sg:(i + 1) * Sg]

            def xc_plane(i):
                return XC[:, i * Sg:(i + 1) * Sg]

            # center + T per tile
            for t in range(NT):
                g, tg = t // 4, t % 4
                lg = int(Lg[g])
                s0 = int(goff[g]) + tg * lg
                for i in range(3):
                    ts(xc_plane(i)[:, s0:s0 + lg], cm_plane(i)[:, s0:s0 + lg],
                       NCEN[:, i * NT + t:i * NT + t + 1], Alu.add)
                ts(TP[:, s0:s0 + lg], xc_plane(0)[:, s0:s0 + lg],
                   V3[:, 0 * NT + t:0 * NT + t + 1], Alu.mult)
                stt(TP[:, s0:s0 + lg], xc_plane(1)[:, s0:s0 + lg],
                    V3[:, 1 * NT + t:1 * NT + t + 1], Alu.mult, Alu.add,
                    TP[:, s0:s0 + lg])
                stt(TP[:, s0:s0 + lg], xc_plane(2)[:, s0:s0 + lg],
                    V3[:, 2 * NT + t:2 * NT + t + 1], Alu.mult, Alu.add,
                    TP[:, s0:s0 + lg])

            # q, r per group; sc per tile
            for g in range(NG):
                lg = int(Lg[g]); s0 = int(goff[g]); w = 4 * lg
                SQ3 = pb.tile([P, 3 * w], dt.float16, tag="SQ3", name=f"SQ3{g}")
                QQ = pb.tile([P, w], dt.float16, tag="QQ", name=f"QQ{g}")
                T2 = pb.tile([P, w], dt.float16, tag="T2", name=f"T2{g}")
                R2 = pb.tile([P, w], dt.float16, tag="R2", name=f"R2{g}")
                RPL = pb.tile([P, w], dt.float16, tag="RPL", name=f"RPL{g}")
                nc.scalar.activation(
                    SQ3[:].rearrange("p (i c) -> p i c", i=3),
                    XC[:].rearrange("p (i c) -> p i c", i=3)[:, :, s0:s0 + w],
                    Act.Square)
                tt(Alu.add, QQ[:], SQ3[:, 0:w], SQ3[:, w:2 * w])
                tt(Alu.add, QQ[:], QQ[:], SQ3[:, 2 * w:3 * w])
                tt(Alu.mult, T2[:], TP[:, s0:s0 + w], TP[:, s0:s0 + w])
                tt(Alu.subtract, R2[:], QQ[:], T2[:])
                ts(R2[:], R2[:], 0.0, Alu.max)
                act(RPL[:], R2[:], Act.Sqrt)
                for tg in range(4):
                    t = 4 * g + tg
                    stt(T2[:, tg * lg:(tg + 1) * lg],
                        TP[:, s0 + tg * lg:s0 + (tg + 1) * lg], 1.0,
                        Alu.mult, Alu.mult,
                        RPL[:, tg * lg:(tg + 1) * lg],
                        accum=SCRAW[:, t:t + 1])

            # ---------------- sign + output --------------------------
            def sign_phase():
                def tmp(tag, k=1):
                    return ret.tile([P, k * NT], dt.float32, tag=tag, name=tag)

                def sl(T, i):
                    return T[:, i * NT:(i + 1) * NT]

                T0 = tmp("T0"); CC = tmp("CC"); R0 = tmp("R0")
                SCV = tmp("SCV"); FAC = tmp("FAC"); SC9 = tmp("SC9")
                GT9 = tmp("GT9"); NPAD = tmp("NPAD")
                tt(Alu.mult, T0[:], sl(CEN, 0), sl(V3, 0))
                tt(Alu.mult, SC9[:], sl(CEN, 1), sl(V3, 1))
                tt(Alu.add, T0[:], T0[:], SC9[:])
                tt(Alu.mult, SC9[:], sl(CEN, 2), sl(V3, 2))
                tt(Alu.add, T0[:], T0[:], SC9[:])
                ts(T0[:], T0[:], -1.0, Alu.mult)
                tt(Alu.mult, CC[:], sl(CEN, 0), sl(CEN, 0))
                tt(Alu.mult, SC9[:], sl(CEN, 1), sl(CEN, 1))
                tt(Alu.add, CC[:], CC[:], SC9[:])
                tt(Alu.mult, SC9[:], sl(CEN, 2), sl(CEN, 2))
                tt(Alu.add, CC[:], CC[:], SC9[:])
                tt(Alu.mult, SC9[:], T0[:], T0[:])
                tt(Alu.subtract, R0[:], CC[:], SC9[:])
                ts(R0[:], R0[:], 0.0, Alu.max)
                act(R0[:], R0[:], Act.Sqrt)
                # padded slots use the group padded length Lg
                for t in range(NT):
                    lg = int(Lg[t // 4])
                    nc.vector.tensor_scalar(
                        out=NPAD[:, t:t + 1],
                        in0=NV[:, t:t + 1], scalar1=-1.0,
                        scalar2=float(lg), op0=Alu.mult, op1=Alu.add)
                tt(Alu.mult, SC9[:], T0[:], R0[:])
                tt(Alu.mult, SC9[:], SC9[:], NPAD[:])
                tt(Alu.subtract, SCV[:], SCRAW[:], SC9[:])
                ts(GT9[:], SCV[:], 0.0, Alu.is_lt)
                nc.vector.tensor_scalar(out=GT9[:], in0=GT9[:], scalar1=-2.0,
                                        scalar2=1.0, op0=Alu.mult, op1=Alu.add)
                tt(Alu.mult, FAC[:], DIRWT[:], GT9[:])
                for i in range(3):
                    tt(Alu.mult, sl(V3, i), sl(V3, i), FAC[:])
                # unscale centers: x16
                for i in range(3):
                    ts(sl(CEN, i), sl(CEN, i), SCL, Alu.mult)
                for j, pl in [(0, sl(CEN, 0)), (1, sl(CEN, 1)), (2, sl(CEN, 2)),
                              (3, sl(B6, 0)), (4, sl(B6, 1)), (5, sl(B6, 2)),
                              (6, sl(B6, 1)), (7, sl(B6, 3)), (8, sl(B6, 4)),
                              (9, sl(B6, 2)), (10, sl(B6, 4)), (11, sl(B6, 5)),
                              (12, sl(V3, 0)), (13, sl(V3, 1)), (14, sl(V3, 2)),
                              (15, NV[:]), (16, MEANV[:]), (17, STDV[:]),
                              (18, MODE[:])]:
                    nc.sync.dma_start(out=res[:, j * NT:(j + 1) * NT], in_=pl)

            sign_phase()
            _pb_stack.close()

    nc.compile()
    return nc


_cache = {}
_last = None


def kernel(data, clust_idx, clust_len):
    global N, C, L, NT, NG
    data = np.asarray(data)
    clust_idx = np.asarray(clust_idx)
    N = int(data.shape[0])
    C, L = int(clust_idx.shape[0]), int(clust_idx.shape[1])
    assert C % (P * N_CORES) == 0
    NT = C // (P * N_CORES)
    NG = NT // 4
    meta = _host_prep(data, clust_idx, clust_len)

    key = (tuple(int(x) for x in meta["Lb"]), N, C)
    if key not in _cache:
        _cache[key] = _build_program(meta)
    nc = _cache[key]

    from concourse.bass_utils import run_bass_kernel_spmd
    in_maps = [{"tstr": meta["tstr"][c], "cstr": meta["cstr"][c],
                "nvec": meta["nvecs"][c]} for c in range(N_CORES)]
    global _last
    _last = (nc, in_maps)
    res = run_bass_kernel_spmd(nc, in_maps, list(range(N_CORES)))

    ids = meta["ids"]
    out = np.zeros((C, 19), dtype=f32)
    for core in range(N_CORES):
        r = res.results[core]["res"].reshape(P, 19, NT)
        for t in range(NT):
            out[ids[core, t]] = r[:, :, t]
    return out
